# revision 7
# baseline (speedup 1.0000x reference)
"""Distributed Trainium2 kernel for the per-agent trajectory attention module.

Math (per reference):
    q = received_messages @ Wq + bq                    [512, 512]
    k = taus @ Wk + bk ; v = taus @ Wv + bv            [16*512, 512/64]
    scores[i, t] = dot(q[i], k[t, i]) / sqrt(512)
    messages[i] = sum_t softmax(scores)[i, t] * v[t, i]  [512, 64]

Key restructure vs the straightforward version: the big k matmul
(taus @ Wk, 2.1 GMAC/core) is eliminated entirely using
    dot(q[i], Wk.T @ tau) == dot(Wk @ q[i], tau)
so after the q reduction each core computes p = Wk @ q_local (a tiny
[2048x512] @ [512x64] product) and scores come from pT.T @ taus directly,
reusing the taus tiles already loaded for v.  bk drops out of softmax
exactly (per-agent constant shift); bv is added at the end.

Sharding over 8 NeuronCores:
  - q: tensor-parallel over the 32768 msg dim (4096 per core).  The
    cross-core reduction runs as two pipelined bf16 AllToAll collectives
    (mesh algorithm, one hop) over the kc-halves of the contraction, plus
    a local DVE tree-sum; the first A2A fires halfway through the q
    accumulation so most of its latency hides under the second half.
  - taus/v/scores/attention: data-parallel over agents (64 per core).
  - All matmul operands bf16; PSUM accumulation f32.
"""

import math

import numpy as np

T = 16
N_AGENTS = 512
TAU = 2048
MSG = 32768
HID = 512
DV = 64

NC = 8
AG = N_AGENTS // NC  # 64 agents per core
MS = MSG // NC  # 4096 msg columns per core
KQ = MS // 128  # 32 msg contraction chunks
HC = HID // 128  # 4 hidden chunks
KT = TAU // 128  # 16 tau contraction chunks
R = T * AG  # 1024 taus rows per core (t-major: r = t*64 + a)

SCALE = 1.0 / math.sqrt(HID)

_CACHE = {}

# set by test harness: run with trace and stash exec time here
TRACE = False
LAST_EXEC_NS = None
LAST_RESULTS = None


def _build():
    import concourse.bacc as bacc
    import concourse.mybir as mybir
    import concourse.tile as tile

    f32 = mybir.dt.float32
    bf16 = mybir.dt.bfloat16
    add = mybir.AluOpType.add
    mult = mybir.AluOpType.mult
    amax = mybir.AluOpType.max

    nc = bacc.Bacc("TRN2", target_bir_lowering=False, debug=False, num_devices=NC)

    # ---- inputs (per-core shards, pre-packed host-side) ----
    # qin: per msg-chunk kc, cols 0:512 = rmT (4 m-tiles of 128 agents),
    # cols 512:1024 = Wq chunk (pre-scaled by 1/sqrt(H))
    qin_d = nc.dram_tensor("qin", [KQ, 128, 1024], bf16, kind="ExternalInput")
    traj_d = nc.dram_tensor("traj", [KT, 128, R], bf16, kind="ExternalInput")
    wkt_d = nc.dram_tensor("wkt", [128, HC, TAU], bf16, kind="ExternalInput")
    wv_d = nc.dram_tensor("wv", [128, KT, DV], bf16, kind="ExternalInput")
    bqs_d = nc.dram_tensor("bqs", [128, HID], f32, kind="ExternalInput")  # bq*SCALE/NC
    bvc_d = nc.dram_tensor("bvc", [DV, 1], f32, kind="ExternalInput")
    idb_d = nc.dram_tensor("idb", [AG, AG], bf16, kind="ExternalInput")  # I_64
    idf_d = nc.dram_tensor("idf", [AG, AG], f32, kind="ExternalInput")  # I_64
    out_d = nc.dram_tensor("out", [AG, DV], f32, kind="ExternalOutput")

    # collective buffers: block p = this core's partial q for core p's agents
    a2a_in = [nc.dram_tensor(f"a2a_in{h}", [N_AGENTS, HID], bf16) for h in range(2)]
    a2a_out = [
        nc.dram_tensor(f"a2a_out{h}", [N_AGENTS, HID], bf16) for h in range(2)
    ]
    alT_d = nc.dram_tensor("alT_d", [T, AG], f32)

    with tile.TileContext(nc) as tc:
        with (
            tc.tile_pool(name="res", bufs=1) as res,
            tc.tile_pool(name="qinp", bufs=4) as qinp,
            tc.tile_pool(name="qdrp", bufs=4) as qdrp,
            tc.tile_pool(name="wrk", bufs=1) as wrk,
            tc.tile_pool(name="qps", bufs=4, space="PSUM") as qps,
            tc.tile_pool(name="svps", bufs=2, space="PSUM") as svps,
        ):
            # ---------------- resident small tensors ----------------
            bqs_sb = res.tile([128, HID], f32)
            bvc_sb = res.tile([DV, 1], f32)
            idb_sb = res.tile([AG, AG], bf16)
            idf_sb = res.tile([AG, AG], f32)
            wv_sb = res.tile([128, KT, DV], bf16)
            nc.scalar.dma_start(bqs_sb[:], bqs_d[:])
            nc.scalar.dma_start(bvc_sb[:], bvc_d[:])
            nc.scalar.dma_start(idb_sb[:], idb_d[:])
            nc.scalar.dma_start(idf_sb[:], idf_d[:])
            nc.scalar.dma_start(wv_sb[:], wv_d[:])

            wkt_sb = res.tile([128, HC, TAU], bf16)
            traj_sb = res.tile([128, KT, R], bf16)

            # ---------------- q phase ----------------
            # qT-major would dodge the transpose but breaks the collective's
            # row sharding; keep agents on PSUM partitions (4 m-tiles).
            qacc = [None] * 4
            for half in range(2):
                for kc_l in range(KQ // 2):
                    kc = half * (KQ // 2) + kc_l
                    qin_t = qinp.tile([128, 1024], bf16, tag="qin", name=f"qin{kc}")
                    nc.sync.dma_start(qin_t[:], qin_d[kc])
                    for m in range(4):
                        if kc_l == 0:
                            qacc[m] = qps.tile(
                                [128, HID], f32, tag="acc", name=f"qacc{half}_{m}"
                            )
                        nc.tensor.matmul(
                            qacc[m][:],
                            qin_t[:, m * 128 : (m + 1) * 128],
                            qin_t[:, 512:1024],
                            start=(kc_l == 0),
                            stop=(kc_l == KQ // 2 - 1),
                        )
                # drain the half: +bq*SCALE/NC once (on half 0), cast bf16
                for m in range(4):
                    qdr = qdrp.tile([128, HID], bf16, tag="qdr", name=f"qdr{half}_{m}")
                    if half == 0:
                        nc.vector.scalar_tensor_tensor(
                            qdr[:], qacc[m][:], 1.0, bqs_sb[:], mult, add
                        )
                    else:
                        nc.vector.tensor_copy(qdr[:], qacc[m][:])
                    dma_eng = nc.scalar if half == 0 else nc.gpsimd
                    dma_eng.dma_start(
                        a2a_in[half][m * 128 : (m + 1) * 128, :], qdr[:]
                    )
                nc.gpsimd.collective_compute(
                    "AllToAll",
                    mybir.AluOpType.bypass,
                    replica_groups=[list(range(NC))],
                    ins=[a2a_in[half].ap().opt()],
                    outs=[a2a_out[half].ap().opt()],
                )

            # ---- DMA issue order on the big queue: wkt then taus chunks ----
            nc.sync.dma_start(wkt_sb[:], wkt_d[:])
            for kc in range(KT):
                nc.sync.dma_start(traj_sb[:, kc, :], traj_d[kc])

            # ---------------- vT during the collective window ----------------
            # sv tiles: partitions 0:64 accumulate scoresT (later), 64:128 vT.
            sv = [svps.tile([128, 512], f32, tag="sv", name=f"sv{h2}") for h2 in range(2)]
            for kc in range(KT):
                for h2 in range(2):
                    nc.tensor.matmul(
                        sv[h2][64:128, :],
                        wv_sb[:, kc, :],
                        traj_sb[:, kc, h2 * 512 : (h2 + 1) * 512],
                        start=(kc == 0),
                        stop=(kc == KT - 1),
                    )

            # ---------------- collective readback + reduce ----------------
            qp = []
            for half in range(2):
                t = res.tile([AG, NC, HID], bf16, name=f"qp{half}")
                nc.scalar.dma_start(
                    t[:], a2a_out[half].ap().rearrange("(b a) h -> a b h", b=NC)
                )
                qp.append(t)
            s0 = wrk.tile([AG, 4, HID], f32, name="s0")
            s1 = wrk.tile([AG, 4, HID], f32, name="s1")
            nc.vector.tensor_tensor(s0[:], qp[0][:, 0:4, :], qp[0][:, 4:8, :], add)
            nc.vector.tensor_tensor(s1[:], qp[1][:, 0:4, :], qp[1][:, 4:8, :], add)
            nc.vector.tensor_tensor(s0[:], s0[:], s1[:], add)
            s2 = wrk.tile([AG, 2, HID], f32, name="s2")
            nc.vector.tensor_tensor(s2[:], s0[:, 0:2, :], s0[:, 2:4, :], add)
            qloc = res.tile([AG, HID], bf16, name="qloc")
            nc.vector.tensor_tensor(qloc[:], s2[:, 0, :], s2[:, 1, :], add)

            # qT via PE transposes: [64, 512] -> [128, 4hc, 64]
            qT_sb = res.tile([128, HC, AG], bf16, name="qT")
            for hc in range(HC):
                tp = qps.tile([128, AG], bf16, tag="sm", bufs=2, name=f"tq{hc}")
                nc.tensor.transpose(
                    tp[:], qloc[:, hc * 128 : (hc + 1) * 128], idb_sb[:]
                )
                nc.vector.tensor_copy(qT_sb[:, hc, :], tp[:])

            # ---------------- pT = Wk @ q_local ----------------
            pT_sb = res.tile([128, KT, AG], bf16, name="pT")
            for kc in range(KT):
                pp = qps.tile([128, AG], f32, tag="sm", bufs=2, name=f"pp{kc}")
                for hc in range(HC):
                    nc.tensor.matmul(
                        pp[:],
                        wkt_sb[:, hc, kc * 128 : (kc + 1) * 128],
                        qT_sb[:, hc, :],
                        start=(hc == 0),
                        stop=(hc == HC - 1),
                    )
                nc.vector.tensor_copy(pT_sb[:, kc, :], pp[:])

            # ---------------- scoresT = pT.T @ taus ----------------
            for kc in range(KT):
                for h2 in range(2):
                    nc.tensor.matmul(
                        sv[h2][0:AG, :],
                        pT_sb[:, kc, :],
                        traj_sb[:, kc, h2 * 512 : (h2 + 1) * 512],
                        start=(kc == 0),
                        stop=(kc == KT - 1),
                    )

            # ---------------- extract scores[a, t] ----------------
            # scoresT[a, r] valid at r = t*64 + a: mask by I_64 over (a, a'),
            # then sum out a'.
            tmpE = wrk.tile([AG, T, AG], f32, name="tmpE")
            for h2 in range(2):
                nc.vector.scalar_tensor_tensor(
                    tmpE[:, h2 * 8 : (h2 + 1) * 8, :],
                    sv[h2][0:AG, :].rearrange("a (t x) -> a t x", t=8),
                    1.0,
                    idf_sb[:].unsqueeze(1).broadcast_to([AG, 8, AG]),
                    mult,
                    mult,
                )
            scores = wrk.tile([AG, T], f32, name="scores")
            nc.vector.tensor_reduce(scores[:], tmpE[:], mybir.AxisListType.X, add)

            # ---------------- softmax over t ----------------
            negmax = wrk.tile([AG, 1], f32, name="negmax")
            nc.vector.tensor_reduce(
                negmax[:], scores[:], mybir.AxisListType.X, amax, negate=True
            )
            ex = wrk.tile([AG, T], f32, name="ex")
            sume = wrk.tile([AG, 1], f32, name="sume")
            nc.scalar.activation(
                ex[:],
                scores[:],
                mybir.ActivationFunctionType.Exp,
                bias=negmax[:],
                accum_out=sume[:],
            )
            rcp = wrk.tile([AG, 1], f32, name="rcp")
            nc.vector.reciprocal(rcp[:], sume[:])
            al_bf = wrk.tile([AG, T], bf16, name="al_bf")
            nc.vector.tensor_scalar_mul(al_bf[:], ex[:], rcp[:])

            # alpha -> flat row-order weights, broadcast to the 64 dv partitions
            alT_ps = qps.tile([T, AG], bf16, tag="sm", bufs=2, name="alT")
            nc.tensor.transpose(alT_ps[:], al_bf[:], idb_sb[:])
            alT_sb = wrk.tile([T, AG], f32, name="alT_sb")
            nc.vector.tensor_copy(alT_sb[:], alT_ps[:])
            nc.scalar.dma_start(alT_d[:], alT_sb[:])
            alw = wrk.tile([DV, T, AG], f32, name="alw")
            nc.scalar.dma_start(
                alw[:], alT_d.ap().unsqueeze(0).broadcast_to([DV, T, AG])
            )

            # ---------------- weighted sum of v ----------------
            tmpW = wrk.tile([DV, R], f32, name="tmpW")
            for h2 in range(2):
                nc.vector.scalar_tensor_tensor(
                    tmpW[:, h2 * 512 : (h2 + 1) * 512],
                    sv[h2][64:128, :],
                    1.0,
                    alw[:].rearrange("d t a -> d (t a)")[
                        :, h2 * 512 : (h2 + 1) * 512
                    ],
                    mult,
                    mult,
                )
            mT = wrk.tile([DV, AG], f32, name="mT")
            nc.vector.tensor_reduce(
                mT[:],
                tmpW[:].rearrange("d (t a) -> d a t", t=T),
                mybir.AxisListType.X,
                add,
            )
            nc.vector.tensor_scalar_add(mT[:], mT[:], bvc_sb[:])

            m_ps = qps.tile([AG, DV], f32, tag="sm", bufs=2, name="m_ps")
            nc.tensor.transpose(m_ps[:], mT[:], idf_sb[:])
            mfin = wrk.tile([AG, DV], f32, name="mfin")
            nc.vector.tensor_copy(mfin[:], m_ps[:])
            nc.scalar.dma_start(out_d[:], mfin[:])

    nc.compile()
    return nc


def _bf(a):
    import ml_dtypes

    return np.ascontiguousarray(a, dtype=ml_dtypes.bfloat16)


def _make_in_maps(imagined_trajectory, received_messages, Wq, bq, Wk, bk, Wv, bv):
    imagined_trajectory = np.asarray(imagined_trajectory, dtype=np.float32)
    received_messages = np.asarray(received_messages, dtype=np.float32)
    Wq = np.asarray(Wq, dtype=np.float32)
    bq = np.asarray(bq, dtype=np.float32)
    Wk = np.asarray(Wk, dtype=np.float32)
    Wv = np.asarray(Wv, dtype=np.float32)
    bv = np.asarray(bv, dtype=np.float32)

    wkt = _bf(Wk.T.reshape(HC, 128, TAU).transpose(1, 0, 2))  # [128, hc, tau]
    wv_p = _bf(Wv.reshape(KT, 128, DV).transpose(1, 0, 2))  # [128, kc, dv]
    bqs = np.ascontiguousarray(
        np.broadcast_to(bq * SCALE / NC, (128, HID)), dtype=np.float32
    )
    bvc = np.ascontiguousarray(bv.reshape(DV, 1), dtype=np.float32)
    idb = _bf(np.eye(AG, dtype=np.float32))
    idf = np.ascontiguousarray(np.eye(AG, dtype=np.float32))

    in_maps = []
    for c in range(NC):
        msl = slice(c * MS, (c + 1) * MS)
        # qin: [128, KQ, 1024]: cols 0:512 rmT (agents), 512:1024 Wq*SCALE
        rmT = received_messages[:, msl].T.reshape(KQ, 128, N_AGENTS)
        wq_sh = (Wq[msl, :] * SCALE).reshape(KQ, 128, HID)
        qin = np.concatenate([rmT, wq_sh], axis=2)
        taus = imagined_trajectory[:, c * AG : (c + 1) * AG, :].reshape(R, TAU)
        traj_p = taus.T.reshape(KT, 128, R)
        in_maps.append(
            {
                "qin": _bf(qin),
                "traj": _bf(traj_p),
                "wkt": wkt,
                "wv": wv_p,
                "bqs": bqs,
                "bvc": bvc,
                "idb": idb,
                "idf": idf,
            }
        )
    return in_maps


def kernel(
    imagined_trajectory,
    received_messages,
    Wq,
    bq,
    Wk,
    bk,
    Wv,
    bv,
):
    global LAST_EXEC_NS, LAST_RESULTS
    from concourse.bass_utils import run_bass_kernel_spmd

    if "nc" not in _CACHE:
        _CACHE["nc"] = _build()
    nc = _CACHE["nc"]

    in_maps = _make_in_maps(
        imagined_trajectory, received_messages, Wq, bq, Wk, bk, Wv, bv
    )

    res = run_bass_kernel_spmd(
        nc,
        in_maps,
        core_ids=list(range(NC)),
        trace=TRACE,
        trace_cores=None,
    )
    LAST_EXEC_NS = res.exec_time_ns
    LAST_RESULTS = res
    out = np.concatenate([res.results[c]["out"] for c in range(NC)], axis=0)
    return np.ascontiguousarray(out, dtype=np.float32)


# revision 8
# speedup vs baseline: 1.0290x; 1.0290x over previous
"""Distributed Trainium2 kernel for the per-agent trajectory attention module.

Math (per reference):
    q = received_messages @ Wq + bq                    [512, 512]
    k = taus @ Wk + bk ; v = taus @ Wv + bv            [16*512, 512/64]
    scores[i, t] = dot(q[i], k[t, i]) / sqrt(512)
    messages[i] = sum_t softmax(scores)[i, t] * v[t, i]  [512, 64]

Key restructure vs the straightforward version: the big k matmul
(taus @ Wk, 2.1 GMAC/core) is eliminated entirely using
    dot(q[i], Wk.T @ tau) == dot(Wk @ q[i], tau)
so after the q reduction each core computes p = Wk @ q_local (a tiny
[2048x512] @ [512x64] product) and scores come from pT.T @ taus directly,
reusing the taus tiles already loaded for v.  bk drops out of softmax
exactly (per-agent constant shift); bv is added at the end.

Sharding over 8 NeuronCores:
  - q: tensor-parallel over the 32768 msg dim (4096 per core).  The
    cross-core reduction runs as two pipelined bf16 AllToAll collectives
    (mesh algorithm, one hop) over the kc-halves of the contraction, plus
    a local DVE tree-sum; the first A2A fires halfway through the q
    accumulation so most of its latency hides under the second half.
  - taus/v/scores/attention: data-parallel over agents (64 per core).
  - All matmul operands bf16; PSUM accumulation f32.
"""

import math

import numpy as np

T = 16
N_AGENTS = 512
TAU = 2048
MSG = 32768
HID = 512
DV = 64

NC = 8
AG = N_AGENTS // NC  # 64 agents per core
MS = MSG // NC  # 4096 msg columns per core
KQ = MS // 128  # 32 msg contraction chunks
HC = HID // 128  # 4 hidden chunks
KT = TAU // 128  # 16 tau contraction chunks
R = T * AG  # 1024 taus rows per core (t-major: r = t*64 + a)

SCALE = 1.0 / math.sqrt(HID)

_CACHE = {}

# set by test harness: run with trace and stash exec time here
TRACE = False
LAST_EXEC_NS = None
LAST_RESULTS = None


def _build():
    import concourse.bacc as bacc
    import concourse.mybir as mybir
    import concourse.tile as tile

    f32 = mybir.dt.float32
    bf16 = mybir.dt.bfloat16
    add = mybir.AluOpType.add
    mult = mybir.AluOpType.mult
    amax = mybir.AluOpType.max

    nc = bacc.Bacc("TRN2", target_bir_lowering=False, debug=False, num_devices=NC)

    # ---- inputs (per-core shards, pre-packed host-side) ----
    # qin: per msg-chunk kc, cols 0:512 = rmT (4 m-tiles of 128 agents),
    # cols 512:1024 = Wq chunk (pre-scaled by 1/sqrt(H))
    qin_d = nc.dram_tensor("qin", [KQ, 128, 1024], bf16, kind="ExternalInput")
    traj_d = nc.dram_tensor("traj", [KT, 128, R], bf16, kind="ExternalInput")
    wkt_d = nc.dram_tensor("wkt", [128, HC, TAU], bf16, kind="ExternalInput")
    wv_d = nc.dram_tensor("wv", [128, KT, DV], bf16, kind="ExternalInput")
    bqs_d = nc.dram_tensor("bqs", [128, HID], f32, kind="ExternalInput")  # bq*SCALE/NC
    bvc_d = nc.dram_tensor("bvc", [DV, 1], f32, kind="ExternalInput")
    idb_d = nc.dram_tensor("idb", [AG, AG], bf16, kind="ExternalInput")  # I_64
    idf_d = nc.dram_tensor("idf", [AG, AG], f32, kind="ExternalInput")  # I_64
    out_d = nc.dram_tensor("out", [AG, DV], f32, kind="ExternalOutput")

    # collective buffers: block p = this core's partial q for core p's agents
    a2a_in = [nc.dram_tensor(f"a2a_in{h}", [N_AGENTS, HID], bf16) for h in range(2)]
    a2a_out = [
        nc.dram_tensor(f"a2a_out{h}", [N_AGENTS, HID], bf16) for h in range(2)
    ]
    alT_d = nc.dram_tensor("alT_d", [T, AG], f32)

    with tile.TileContext(nc) as tc:
        with (
            tc.tile_pool(name="res", bufs=1) as res,
            tc.tile_pool(name="qinp", bufs=4) as qinp,
            tc.tile_pool(name="qdrp", bufs=4) as qdrp,
            tc.tile_pool(name="wrk", bufs=1) as wrk,
            tc.tile_pool(name="qps", bufs=4, space="PSUM") as qps,
            tc.tile_pool(name="svps", bufs=2, space="PSUM") as svps,
        ):
            # ---------------- resident small tensors ----------------
            bqs_sb = res.tile([128, HID], f32)
            bvc_sb = res.tile([DV, 1], f32)
            idb_sb = res.tile([AG, AG], bf16)
            idf_sb = res.tile([AG, AG], f32)
            wv_sb = res.tile([128, KT, DV], bf16)
            nc.scalar.dma_start(bqs_sb[:], bqs_d[:])
            nc.scalar.dma_start(bvc_sb[:], bvc_d[:])
            nc.scalar.dma_start(idb_sb[:], idb_d[:])
            nc.scalar.dma_start(idf_sb[:], idf_d[:])
            nc.scalar.dma_start(wv_sb[:], wv_d[:])

            wkt_sb = res.tile([128, HC, TAU], bf16)
            traj_sb = res.tile([128, KT, R], bf16)

            # ---------------- q phase ----------------
            qacc = [None] * 4
            for g in range(KQ // 4):
                qin_t = qinp.tile([128, 4, 1024], bf16, tag="qin", name=f"qin{g}")
                nc.sync.dma_start(
                    qin_t[:], qin_d.ap()[4 * g : 4 * g + 4].rearrange("k p x -> p k x")
                )
                for kl in range(4):
                    kc = 4 * g + kl
                    for m in range(4):
                        if kc == 0:
                            qacc[m] = qps.tile(
                                [128, HID], f32, tag="acc", name=f"qacc{m}"
                            )
                        nc.tensor.matmul(
                            qacc[m][:],
                            qin_t[:, kl, m * 128 : (m + 1) * 128],
                            qin_t[:, kl, 512:1024],
                            start=(kc == 0),
                            stop=(kc == KQ - 1),
                        )
            for m in range(4):
                qdr = qdrp.tile([128, HID], bf16, tag="qdr", name=f"qdr{m}")
                nc.vector.scalar_tensor_tensor(
                    qdr[:], qacc[m][:], 1.0, bqs_sb[:], mult, add
                )
                nc.scalar.dma_start(a2a_in[0][m * 128 : (m + 1) * 128, :], qdr[:])
            nc.gpsimd.collective_compute(
                "AllToAll",
                mybir.AluOpType.bypass,
                replica_groups=[list(range(NC))],
                ins=[a2a_in[0].ap().opt()],
                outs=[a2a_out[0].ap().opt()],
            )

            # ---- DMA issue order on the big queue: wkt then taus chunks ----
            nc.sync.dma_start(wkt_sb[:], wkt_d[:])
            for g in range(KT // 4):
                nc.sync.dma_start(
                    traj_sb[:, 4 * g : 4 * g + 4, :],
                    traj_d.ap()[4 * g : 4 * g + 4].rearrange("k p x -> p k x"),
                )

            # ---------------- vT during the collective window ----------------
            # sv tiles: partitions 0:64 accumulate scoresT (later), 64:128 vT.
            sv = [svps.tile([128, 512], f32, tag="sv", name=f"sv{h2}") for h2 in range(2)]
            for kc in range(KT):
                for h2 in range(2):
                    nc.tensor.matmul(
                        sv[h2][64:128, :],
                        wv_sb[:, kc, :],
                        traj_sb[:, kc, h2 * 512 : (h2 + 1) * 512],
                        start=(kc == 0),
                        stop=(kc == KT - 1),
                    )

            # ---------------- collective readback + reduce ----------------
            qp0 = res.tile([AG, NC, HID], bf16, name="qp0")
            nc.scalar.dma_start(
                qp0[:], a2a_out[0].ap().rearrange("(b a) h -> a b h", b=NC)
            )
            s0 = wrk.tile([AG, 4, HID], bf16, name="s0")
            nc.vector.tensor_tensor(s0[:], qp0[:, 0:4, :], qp0[:, 4:8, :], add)
            s2 = wrk.tile([AG, 2, HID], bf16, name="s2")
            nc.vector.tensor_tensor(s2[:], s0[:, 0:2, :], s0[:, 2:4, :], add)
            qloc = res.tile([AG, HID], bf16, name="qloc")
            nc.vector.tensor_tensor(qloc[:], s2[:, 0, :], s2[:, 1, :], add)

            # qT via PE transposes: [64, 512] -> [128, 4hc, 64]
            qT_sb = res.tile([128, HC, AG], bf16, name="qT")
            for hc in range(HC):
                tp = qps.tile([128, AG], bf16, tag="sm", bufs=2, name=f"tq{hc}")
                nc.tensor.transpose(
                    tp[:], qloc[:, hc * 128 : (hc + 1) * 128], idb_sb[:]
                )
                nc.vector.tensor_copy(qT_sb[:, hc, :], tp[:])

            # ---------------- pT = Wk @ q_local ----------------
            pT_sb = res.tile([128, KT, AG], bf16, name="pT")
            for kc in range(KT):
                pp = qps.tile([128, AG], f32, tag="sm", bufs=2, name=f"pp{kc}")
                for hc in range(HC):
                    nc.tensor.matmul(
                        pp[:],
                        wkt_sb[:, hc, kc * 128 : (kc + 1) * 128],
                        qT_sb[:, hc, :],
                        start=(hc == 0),
                        stop=(hc == HC - 1),
                    )
                nc.vector.tensor_copy(pT_sb[:, kc, :], pp[:])

            # ---------------- scoresT = pT.T @ taus ----------------
            for kc in range(KT):
                for h2 in range(2):
                    nc.tensor.matmul(
                        sv[h2][0:AG, :],
                        pT_sb[:, kc, :],
                        traj_sb[:, kc, h2 * 512 : (h2 + 1) * 512],
                        start=(kc == 0),
                        stop=(kc == KT - 1),
                    )

            # ---------------- extract scores[a, t] ----------------
            # scoresT[a, r] valid at r = t*64 + a: mask by I_64 over (a, a'),
            # then sum out a'.
            tmpE = wrk.tile([AG, T, AG], f32, name="tmpE")
            for h2 in range(2):
                nc.vector.scalar_tensor_tensor(
                    tmpE[:, h2 * 8 : (h2 + 1) * 8, :],
                    sv[h2][0:AG, :].rearrange("a (t x) -> a t x", t=8),
                    1.0,
                    idf_sb[:].unsqueeze(1).broadcast_to([AG, 8, AG]),
                    mult,
                    mult,
                )
            scores = wrk.tile([AG, T], f32, name="scores")
            nc.vector.tensor_reduce(scores[:], tmpE[:], mybir.AxisListType.X, add)

            # ---------------- softmax over t ----------------
            negmax = wrk.tile([AG, 1], f32, name="negmax")
            nc.vector.tensor_reduce(
                negmax[:], scores[:], mybir.AxisListType.X, amax, negate=True
            )
            ex = wrk.tile([AG, T], f32, name="ex")
            sume = wrk.tile([AG, 1], f32, name="sume")
            nc.scalar.activation(
                ex[:],
                scores[:],
                mybir.ActivationFunctionType.Exp,
                bias=negmax[:],
                accum_out=sume[:],
            )
            rcp = wrk.tile([AG, 1], f32, name="rcp")
            nc.vector.reciprocal(rcp[:], sume[:])
            al_bf = wrk.tile([AG, T], bf16, name="al_bf")
            nc.vector.tensor_scalar_mul(al_bf[:], ex[:], rcp[:])

            # alpha -> flat row-order weights, broadcast to the 64 dv partitions
            alT_ps = qps.tile([T, AG], bf16, tag="sm", bufs=2, name="alT")
            nc.tensor.transpose(alT_ps[:], al_bf[:], idb_sb[:])
            alT_sb = wrk.tile([T, AG], f32, name="alT_sb")
            nc.vector.tensor_copy(alT_sb[:], alT_ps[:])
            nc.scalar.dma_start(alT_d[:], alT_sb[:])
            alw = wrk.tile([DV, T, AG], f32, name="alw")
            nc.scalar.dma_start(
                alw[:], alT_d.ap().unsqueeze(0).broadcast_to([DV, T, AG])
            )

            # ---------------- weighted sum of v ----------------
            tmpW = wrk.tile([DV, R], f32, name="tmpW")
            for h2 in range(2):
                nc.vector.scalar_tensor_tensor(
                    tmpW[:, h2 * 512 : (h2 + 1) * 512],
                    sv[h2][64:128, :],
                    1.0,
                    alw[:].rearrange("d t a -> d (t a)")[
                        :, h2 * 512 : (h2 + 1) * 512
                    ],
                    mult,
                    mult,
                )
            mT = wrk.tile([DV, AG], f32, name="mT")
            nc.vector.tensor_reduce(
                mT[:],
                tmpW[:].rearrange("d (t a) -> d a t", t=T),
                mybir.AxisListType.X,
                add,
            )
            nc.vector.tensor_scalar_add(mT[:], mT[:], bvc_sb[:])

            m_ps = qps.tile([AG, DV], f32, tag="sm", bufs=2, name="m_ps")
            nc.tensor.transpose(m_ps[:], mT[:], idf_sb[:])
            mfin = wrk.tile([AG, DV], f32, name="mfin")
            nc.vector.tensor_copy(mfin[:], m_ps[:])
            nc.scalar.dma_start(out_d[:], mfin[:])

    nc.compile()
    return nc


def _bf(a):
    import ml_dtypes

    return np.ascontiguousarray(a, dtype=ml_dtypes.bfloat16)


def _make_in_maps(imagined_trajectory, received_messages, Wq, bq, Wk, bk, Wv, bv):
    imagined_trajectory = np.asarray(imagined_trajectory, dtype=np.float32)
    received_messages = np.asarray(received_messages, dtype=np.float32)
    Wq = np.asarray(Wq, dtype=np.float32)
    bq = np.asarray(bq, dtype=np.float32)
    Wk = np.asarray(Wk, dtype=np.float32)
    Wv = np.asarray(Wv, dtype=np.float32)
    bv = np.asarray(bv, dtype=np.float32)

    wkt = _bf(Wk.T.reshape(HC, 128, TAU).transpose(1, 0, 2))  # [128, hc, tau]
    wv_p = _bf(Wv.reshape(KT, 128, DV).transpose(1, 0, 2))  # [128, kc, dv]
    bqs = np.ascontiguousarray(
        np.broadcast_to(bq * SCALE / NC, (128, HID)), dtype=np.float32
    )
    bvc = np.ascontiguousarray(bv.reshape(DV, 1), dtype=np.float32)
    idb = _bf(np.eye(AG, dtype=np.float32))
    idf = np.ascontiguousarray(np.eye(AG, dtype=np.float32))

    in_maps = []
    for c in range(NC):
        msl = slice(c * MS, (c + 1) * MS)
        # qin: [128, KQ, 1024]: cols 0:512 rmT (agents), 512:1024 Wq*SCALE
        rmT = received_messages[:, msl].T.reshape(KQ, 128, N_AGENTS)
        wq_sh = (Wq[msl, :] * SCALE).reshape(KQ, 128, HID)
        qin = np.concatenate([rmT, wq_sh], axis=2)
        taus = imagined_trajectory[:, c * AG : (c + 1) * AG, :].reshape(R, TAU)
        traj_p = taus.T.reshape(KT, 128, R)
        in_maps.append(
            {
                "qin": _bf(qin),
                "traj": _bf(traj_p),
                "wkt": wkt,
                "wv": wv_p,
                "bqs": bqs,
                "bvc": bvc,
                "idb": idb,
                "idf": idf,
            }
        )
    return in_maps


def kernel(
    imagined_trajectory,
    received_messages,
    Wq,
    bq,
    Wk,
    bk,
    Wv,
    bv,
):
    global LAST_EXEC_NS, LAST_RESULTS
    from concourse.bass_utils import run_bass_kernel_spmd

    if "nc" not in _CACHE:
        _CACHE["nc"] = _build()
    nc = _CACHE["nc"]

    in_maps = _make_in_maps(
        imagined_trajectory, received_messages, Wq, bq, Wk, bk, Wv, bv
    )

    res = run_bass_kernel_spmd(
        nc,
        in_maps,
        core_ids=list(range(NC)),
        trace=TRACE,
        trace_cores=None,
    )
    LAST_EXEC_NS = res.exec_time_ns
    LAST_RESULTS = res
    out = np.concatenate([res.results[c]["out"] for c in range(NC)], axis=0)
    return np.ascontiguousarray(out, dtype=np.float32)
